# revision 1
# baseline (speedup 1.0000x reference)
"""GCN layer kernel for Trainium2 (8 NeuronCores, SPMD).

Computes: support = x @ W.T + b; agg[d] = sum_{e: dst[e]=d} val[e] * support[src[e]];
out = leaky_relu(agg, 0.2).

Strategy (dst-sharded, 6250 dst nodes / ~100K edges per core):
  Phase 1: every core computes the full support [N, 64] f32 (x fed as
           pre-transposed bf16) and writes it to HBM in per-group DMAs.
           The HBM layout is a group-aligned partition-major permutation
           (see _row_of_node) so each write is contiguous per partition
           (full DMA rate, no sub-512B penalty).
  Phase 2: edges are packed on the host into 128-edge chunks grouped under
           64-dst windows with uniform chunks-per-window counts. dma_gather
           uses int16 row indices (max 32767), so edges go to two gather
           streams: "A" from base row 0 (rows < 32768) and "B" from base
           row n_rows - 32768. The ranges overlap, so edges whose row lies
           in the overlap are assigned per window to balance the two
           streams' chunk caps (minimizing pad slots). Per chunk: a
           one-hot "S" matrix
           ([128 edges x 64 dsts], S[p, dcol[p]] = val[p]) built on the
           vector engine with one fused is_equal*val tensor_scalar op,
           then a PE matmul S.T @ G accumulated in PSUM over the window.
           Window pairs share one PSUM tile (partitions 0-63 / 64-127).
           Leaky relu is applied at PSUM flush; output is staged in SBUF
           (partition-major) and written with one DMA, de-permuted on the
           host.
"""

import math

import numpy as np
import ml_dtypes

N_NODES = 50000
N_EDGES = 800000
D_IN = 256
D_OUT = 64
NEG_SLOPE = 0.2
N_CORES = 8
NPC = N_NODES // N_CORES  # dst nodes per core
WIN = 64  # dst window width (S matrix columns)
SPLIT = 32768  # int16 index limit: A stream < SPLIT, B stream >= SPLIT
MG_CHUNKS = 32  # phase-1 node chunks per xT load group
GC = 10  # chunks per dma_gather call
GBUFS = 12  # gather tile buffer depth
SBUFS = 16  # S-matrix buffer depth
XBUFS = 3  # xT tile buffer depth
PSBUFS = 8  # unified PSUM pool depth
FBUFS = 4  # flush tmp buffer depth
SGBUFS = 2  # phase-1 support group buffer depth
PHASE_ONLY = 0  # debug: 1 = phase-1 only, 2 = phase-2 only (support external)


def _wrap16(a):
    """[?, n] -> int16 indices wrapped: position j -> [j % 16, j // 16]."""
    n = a.shape[-1]
    assert n % 16 == 0
    return np.ascontiguousarray(
        a.reshape(*a.shape[:-1], n // 16, 16).swapaxes(-1, -2)
    )


def _row_of_node(n_sup_chunks):
    """Group-aligned partition-major support row permutation.

    Within each MG_CHUNKS-chunk phase-1 group, node n maps to row
    base + (n % 128) * gsz + (n // 128 - first_chunk), so every group's
    write is contiguous per partition AND group g occupies exactly rows
    [base_g, base_g + gsz * 128) -- letting the support table be split
    into two HBM tensors at a group boundary.
    """
    row = np.empty(n_sup_chunks * 128, np.int64)
    done = 0
    while done < n_sup_chunks:
        g = min(MG_CHUNKS, n_sup_chunks - done)
        nodes = np.arange(done * 128, (done + g) * 128)
        row[nodes] = done * 128 + (nodes % 128) * g + (nodes // 128 - done)
        done += g
    return row


def _build_schedule(edge_src, edge_dst, edge_val):
    """Pack edges into per-core per-stream slot arrays.

    Returns dict with idx/dcol/val arrays for A and B streams
    ([N_CORES, 128, nch_s] each) and (nchw_a, nchw_b, nwin).
    """
    nwin = -(-NPC // WIN)  # windows per core
    n_sup_chunks = -(-N_NODES // 128)
    core_bounds = np.searchsorted(edge_dst, np.arange(0, N_NODES + 1, NPC))

    per_core = []
    nchw = [1, 1]
    for k in range(N_CORES):
        lo, hi = int(core_bounds[k]), int(core_bounds[k + 1])
        src_n = edge_src[lo:hi]
        src_k = _row_of_node(n_sup_chunks)[src_n]
        dloc = edge_dst[lo:hi].astype(np.int64) - k * NPC
        val_k = edge_val[lo:hi]
        w = dloc // WIN
        # Stream A gathers rows [0, SPLIT); stream B gathers rows
        # [base1, base1 + SPLIT) where base1 = n_rows - SPLIT. Rows in
        # [base1, SPLIT) fit either stream; assign them per window to
        # balance the two streams' chunk caps (A up to A_CAP slots).
        n_rows = n_sup_chunks * 128
        base1 = max(0, n_rows - SPLIT)
        a_cap = 640 if base1 < SPLIT else 10**9
        aonly = src_k < base1
        conly = src_k >= SPLIT
        flex = ~aonly & ~conly
        cnt_a = np.bincount(w[aonly], minlength=nwin)
        cnt_f = np.bincount(w[flex], minlength=nwin)
        # flexible edges taken by A per window
        take = np.minimum(cnt_a + cnt_f, a_cap) - cnt_a
        take = np.maximum(take, 0)
        # ordinal of each flexible edge within its window's flexible list
        fw = w[flex]
        start_f = np.zeros(nwin, np.int64)
        start_f[1:] = np.cumsum(cnt_f)[:-1]
        ordinal = np.arange(flex.sum(), dtype=np.int64) - start_f[fw]
        flex_to_a = np.zeros(len(src_k), bool)
        flex_to_a[np.where(flex)[0]] = ordinal < take[fw]
        in_a = aonly | flex_to_a
        streams = []
        for s, mask in enumerate([in_a, ~in_a]):
            ws = w[mask]
            counts = np.bincount(ws, minlength=nwin)
            nchw[s] = max(nchw[s], int(math.ceil(counts.max() / 128)))
            streams.append(
                (src_k[mask] - s * base1, dloc[mask], val_k[mask], ws, counts)
            )
        per_core.append(streams)

    out = {}
    for s in range(2):
        nch = nwin * nchw[s]
        idx = np.zeros((N_CORES, 128, nch), np.int16)
        dcol = np.zeros((N_CORES, 128, nch), np.float32)
        val = np.zeros((N_CORES, 128, nch), np.float32)
        for k in range(N_CORES):
            srcs, dloc, vals, ws, counts = per_core[k][s]
            win_start = np.zeros(nwin, np.int64)
            win_start[1:] = np.cumsum(counts)[:-1]
            within = np.arange(len(srcs), dtype=np.int64) - win_start[ws]
            slot_c = ws * nchw[s] + within // 128
            slot_p = within % 128
            idx[k, slot_p, slot_c] = srcs
            dcol[k, slot_p, slot_c] = (dloc - ws * WIN).astype(np.float32)
            val[k, slot_p, slot_c] = vals
        out[s] = (idx, dcol, val)
    return out, nchw[0], nchw[1], nwin


def _build_program(nchw_a, nchw_b, nwin, n_sup_chunks):
    import concourse.bacc as bacc
    import concourse.mybir as mybir
    from concourse import tile

    F32 = mybir.dt.float32
    BF16 = mybir.dt.bfloat16
    I16 = mybir.dt.int16
    nch = [nwin * nchw_a, nwin * nchw_b]
    nwp = nwin // 2  # window pairs
    n_sup_rows = n_sup_chunks * 128

    nc = bacc.Bacc(None, target_bir_lowering=False, debug=False)
    xT_d = nc.dram_tensor("xT", [D_IN, n_sup_rows], BF16, kind="ExternalInput")
    wT_d = nc.dram_tensor("wT", [D_IN, D_OUT], BF16, kind="ExternalInput")
    bbc_d = nc.dram_tensor("bbc", [128, D_OUT], F32, kind="ExternalInput")
    iota_d = nc.dram_tensor("iota", [128, WIN], F32, kind="ExternalInput")
    idx_d = [
        nc.dram_tensor(f"idx{s}", [128, nch[s] * 8], I16, kind="ExternalInput")
        for s in range(2)
    ]
    dcol_d = [
        nc.dram_tensor(f"dcol{s}", [128, nch[s]], F32, kind="ExternalInput")
        for s in range(2)
    ]
    val_d = [
        nc.dram_tensor(f"val{s}", [128, nch[s]], F32, kind="ExternalInput")
        for s in range(2)
    ]
    kind = "ExternalInput" if PHASE_ONLY == 2 else "Internal"
    sup_d = nc.dram_tensor("support", [n_sup_rows, D_OUT], F32, kind=kind)
    y_d = nc.dram_tensor("y", [128, nwp, D_OUT], F32, kind="ExternalOutput")

    with tile.TileContext(nc) as tc:
        with (
            tc.tile_pool(name="const", bufs=1) as cpool,
            tc.tile_pool(name="stage", bufs=1) as stpool,
            tc.tile_pool(name="xt", bufs=XBUFS) as xpool,
            tc.tile_pool(name="gath", bufs=GBUFS) as gpool,
            tc.tile_pool(name="smat", bufs=SBUFS) as spool,
            tc.tile_pool(name="flush", bufs=FBUFS) as fpool,
            tc.tile_pool(name="psum", bufs=PSBUFS, space="PSUM") as pspool,
        ):
            # constants / metadata
            wt_t = cpool.tile([128, 2, D_OUT], BF16)
            bbc_t = cpool.tile([128, D_OUT], F32)
            iota_t = cpool.tile([128, WIN], F32)
            idx_t = [
                cpool.tile([128, nch[s] * 8], I16, name=f"idx{s}t", tag=f"idx{s}") for s in range(2)
            ]
            dcol_t = [
                cpool.tile([128, nch[s]], F32, name=f"dcol{s}t", tag=f"dcol{s}") for s in range(2)
            ]
            val_t = [
                cpool.tile([128, nch[s]], F32, name=f"val{s}t", tag=f"val{s}") for s in range(2)
            ]
            wT_v = wT_d.rearrange("(kk p) f -> p kk f", p=128)
            nc.sync.dma_start(out=wt_t[:], in_=wT_v)
            nc.sync.dma_start(out=bbc_t[:], in_=bbc_d[:])
            nc.sync.dma_start(out=iota_t[:], in_=iota_d[:])
            for s in range(2):
                nc.sync.dma_start(out=idx_t[s][:], in_=idx_d[s][:])
                nc.sync.dma_start(out=dcol_t[s][:], in_=dcol_d[s][:])
                nc.sync.dma_start(out=val_t[s][:], in_=val_d[s][:])

            # phase 1: support = x @ W.T + b, written to HBM per group
            done = 0
            while PHASE_ONLY != 2 and done < n_sup_chunks:
                g = min(MG_CHUNKS, n_sup_chunks - done)
                xt_t = xpool.tile([128, 2, g * 128], BF16, tag="xt")
                sup_grp = stpool.tile(
                    [128, g, D_OUT], F32, tag="supgrp", bufs=SGBUFS
                )
                for kk in range(2):
                    nc.sync.dma_start(
                        out=xt_t[:, kk, :],
                        in_=xT_d[kk * 128 : (kk + 1) * 128,
                                 done * 128 : (done + g) * 128],
                    )
                for j in range(g):
                    p1 = pspool.tile([128, D_OUT], F32, name="p1", tag="ps")
                    for kk in range(2):
                        nc.tensor.matmul(
                            p1[:],
                            xt_t[:, kk, j * 128 : (j + 1) * 128],
                            wt_t[:, kk, :],
                            start=(kk == 0),
                            stop=(kk == 1),
                        )
                    nc.vector.tensor_tensor(
                        sup_grp[:, j, :], p1[:], bbc_t[:],
                        op=mybir.AluOpType.add,
                    )
                off = done * 128
                gv = sup_d[off : off + g * 128, :].rearrange(
                    "(p ch) f -> p ch f", ch=g
                )
                nc.sync.dma_start(out=gv, in_=sup_grp[:])
                done += g

            # phase 2: A/B gathers + S-matmul segment sum
            out_stage = stpool.tile([128, nwp, D_OUT], F32)
            if PHASE_ONLY == 1:
                nc.vector.memset(out_stage[:], 0.0)
            g_tiles = {0: {}, 1: {}}
            base1 = max(0, n_sup_rows - SPLIT)
            sup_base = [
                sup_d[0 : min(SPLIT, n_sup_rows), :],
                sup_d[base1:n_sup_rows, :],
            ]

            def ensure_gather(s, c):
                g = c // GC
                if g not in g_tiles[s]:
                    lo = g * GC
                    hi = min(lo + GC, nch[s])
                    n_idx = (hi - lo) * 128
                    gt = gpool.tile([128, hi - lo, D_OUT], F32, name=f"g{s}_{g}", tag=f"G{s}")
                    nc.gpsimd.dma_gather(
                        gt[:, :, :],
                        sup_base[s],
                        idx_t[s][:, lo * 8 : hi * 8],
                        n_idx,
                        n_idx,
                        D_OUT,
                        single_packet=False,
                    )
                    g_tiles[s][g] = gt
                return g_tiles[s][g], c - g * GC

            for wp in range(nwp if PHASE_ONLY != 1 else 0):
                p2 = pspool.tile([128, D_OUT], F32, name="p2", tag="ps")
                for w01 in range(2):
                    w = wp * 2 + w01
                    n_acc = nchw_a + nchw_b
                    j = 0
                    for s, nchw_s in ((0, nchw_a), (1, nchw_b)):
                        for i in range(nchw_s):
                            c = w * nchw_s + i
                            gt, off = ensure_gather(s, c)
                            s_t = spool.tile([128, WIN], F32, tag="S")
                            nc.vector.tensor_scalar(
                                s_t[:], iota_t[:],
                                dcol_t[s][:, c : c + 1], val_t[s][:, c : c + 1],
                                op0=mybir.AluOpType.is_equal,
                                op1=mybir.AluOpType.mult,
                            )
                            nc.tensor.matmul(
                                p2[64 * w01 : 64 * (w01 + 1), :],
                                s_t[:],
                                gt[:, off, :],
                                start=(j == 0),
                                stop=(j == n_acc - 1),
                            )
                            j += 1
                tmp = fpool.tile([128, D_OUT], F32, tag="tmp")
                nc.scalar.activation(
                    tmp[:], p2[:], mybir.ActivationFunctionType.Copy,
                    scale=NEG_SLOPE,
                )
                nc.vector.tensor_tensor(
                    out_stage[:, wp, :], tmp[:], p2[:], op=mybir.AluOpType.max
                )
            nc.sync.dma_start(out=y_d[:], in_=out_stage[:])
    nc.compile()
    return nc


LAST_RESULTS = None  # BassKernelResults of the most recent run (for profiling)
LAST_NC = None  # compiled Bass module of the most recent run


def kernel(x, W, b, edge_src, edge_dst, edge_val):
    global LAST_RESULTS, LAST_NC
    from concourse import bass_utils

    x = np.asarray(x)
    W = np.asarray(W)
    b = np.asarray(b)
    edge_src = np.asarray(edge_src)
    edge_dst = np.asarray(edge_dst)
    edge_val = np.asarray(edge_val)

    streams, nchw_a, nchw_b, nwin = _build_schedule(edge_src, edge_dst, edge_val)
    n_sup_chunks = -(-N_NODES // 128)
    n_sup_rows = n_sup_chunks * 128

    xT = np.zeros((D_IN, n_sup_rows), ml_dtypes.bfloat16)
    xT[:, :N_NODES] = x.T.astype(ml_dtypes.bfloat16)
    wT = np.ascontiguousarray(W.T).astype(ml_dtypes.bfloat16)
    bbc = np.tile(b.astype(np.float32), (128, 1))
    iota = np.tile(np.arange(WIN, dtype=np.float32), (128, 1))

    nc = _build_program(nchw_a, nchw_b, nwin, n_sup_chunks)
    LAST_NC = nc

    shared = {"xT": xT, "wT": wT, "bbc": bbc, "iota": iota}
    in_maps = []
    for k in range(N_CORES):
        m = dict(shared)
        for s in range(2):
            idx, dcol, val = streams[s]
            nch_s = idx.shape[-1]
            # wrap each chunk's 128 indices: j -> [j % 16, j // 16]
            iw = _wrap16(idx[k].T.reshape(nch_s * 128))  # [16, nch*8]
            m[f"idx{s}"] = np.ascontiguousarray(np.tile(iw, (8, 1)))
            m[f"dcol{s}"] = dcol[k]
            m[f"val{s}"] = val[k]
        in_maps.append(m)

    res = None
    for attempt in range(3):
        try:
            res = bass_utils.run_bass_kernel_spmd(
                nc, in_maps, core_ids=list(range(N_CORES))
            )
            break
        except Exception:
            # Transient NRT/axon execution failures have been observed; the
            # device recovers on a fresh dispatch. Re-raise on the last try.
            if attempt == 2:
                raise
    LAST_RESULTS = res
    out = np.concatenate(
        [
            res.results[k]["y"].transpose(1, 0, 2).reshape(-1, D_OUT)[:NPC]
            for k in range(N_CORES)
        ],
        axis=0,
    )
    return out.astype(np.float32)


if __name__ == "__main__":
    pass



# revision 27
# speedup vs baseline: 1.5541x; 1.5541x over previous
"""GCN layer kernel for Trainium2 (8 NeuronCores, SPMD).

Computes: support = x @ W.T + b; agg[d] = sum_{e: dst[e]=d} val[e] * support[src[e]];
out = leaky_relu(agg, 0.2).

Strategy (dst-sharded, 6250 dst nodes / ~100K edges per core):
  Phase 1: every core computes the full support [N, 64] in BF16 (x fed as
           pre-transposed bf16) and writes it to HBM in per-group DMAs.
           The HBM layout is a group-aligned partition-major permutation
           (see _row_of_node) so each write is contiguous per partition.
  Phase 2: edges are packed on the host into 128-edge chunks grouped under
           128-dst windows. Chunk counts per (window, parity-stream) are the
           max over cores (the program is shared SPMD). The support table is
           viewed as [25024 pairs x 256B]; a raw InstDMAGatherAnt (emitted
           directly to allow elem_size_bytes=128 < the API's 256B minimum;
           the 256B rule is the *stride* granularity, which we satisfy with
           elem_step=128 bf16) gathers the 64-feature bf16 row of each edge:
           stream 0 reads pair offset +0 (even rows), stream 1 offset +64
           elements (odd rows), so int16 pair indices stay < 32768. Per
           chunk: a one-hot "S" matrix ([128 edges x 128 dsts] bf16,
           S[p, dcol[p]] = val[p]) built on the vector engine with one fused
           is_equal*val tensor_scalar op, then a PE matmul S.T @ G
           accumulated in PSUM over the window. Leaky relu is applied at
           PSUM flush; output is staged in SBUF and written with one DMA,
           de-permuted on the host.
"""

import math

import numpy as np
import ml_dtypes

N_NODES = 50000
N_EDGES = 800000
D_IN = 256
D_OUT = 64
NEG_SLOPE = 0.2
N_CORES = 8
NPC = N_NODES // N_CORES  # dst nodes per core
WIN = 128  # dst window width (S matrix columns / psum partitions)
MG_CHUNKS = 16  # phase-1 node chunks per xT load group
GC = 48  # chunks per dma_gather call
GBUFS = 3  # gather tile buffer depth (per stream)
SGRP = 8  # S-matrix chunks per tile (amortizes tile-pool sync)
SBUFS = 6  # S-matrix buffer depth
NPRE = 384  # S-matrix chunks pre-built into a static arena during phase 1
B1 = 16  # phase-1 node chunks per PSUM tile / Act copy
XBUFS = 3  # xT tile buffer depth
P1BUFS = 2  # phase-1 PSUM pool depth ([128, B1*64] f32 tiles)
P2BUFS = 4  # phase-2 PSUM pool depth ([128, 64] f32 tiles)
FBUFS = 4  # flush tmp buffer depth
SGBUFS = 2  # phase-1 support group buffer depth
YSPLIT = 4  # output DMA pieces


def _wrap16(a):
    """[?, n] -> int16 indices wrapped: position j -> [j % 16, j // 16]."""
    n = a.shape[-1]
    assert n % 16 == 0
    return np.ascontiguousarray(
        a.reshape(*a.shape[:-1], n // 16, 16).swapaxes(-1, -2)
    )


def _row_of_node(n_sup_chunks):
    """Group-aligned partition-major support row permutation.

    Within each MG_CHUNKS-chunk phase-1 group, node n maps to row
    base + (n % 128) * gsz + (n // 128 - first_chunk), so every group's
    write is contiguous per partition.
    """
    row = np.empty(n_sup_chunks * 128, np.int64)
    done = 0
    while done < n_sup_chunks:
        g = min(MG_CHUNKS, n_sup_chunks - done)
        nodes = np.arange(done * 128, (done + g) * 128)
        row[nodes] = done * 128 + (nodes % 128) * g + (nodes // 128 - done)
        done += g
    return row


def _build_schedule(edge_src, edge_dst, edge_val):
    """Pack edges into per-core per-parity-stream slot arrays.

    Returns (streams, cnt) where streams[s] = (idx, dcol, val) arrays of
    shape [N_CORES, 128, nch_s] (idx = int16 pair index) and cnt[s] is the
    per-window chunk count list (shared across cores: max over cores).
    """
    nwin = -(-NPC // WIN)  # windows per core
    core_bounds = np.searchsorted(edge_dst, np.arange(0, N_NODES + 1, NPC))

    # per-core used source nodes: each core's support table only holds the
    # ~43K nodes its edges reference (compacted, then group-permuted)
    used = [
        np.unique(edge_src[int(core_bounds[k]) : int(core_bounds[k + 1])])
        for k in range(N_CORES)
    ]
    n_sup_chunks = -(-max(len(u) for u in used) // 128)
    row_perm = _row_of_node(n_sup_chunks)

    per_core = []
    counts = np.zeros((2, N_CORES, nwin), np.int64)
    for k in range(N_CORES):
        lo, hi = int(core_bounds[k]), int(core_bounds[k + 1])
        rows = row_perm[np.searchsorted(used[k], edge_src[lo:hi])]
        pair = rows >> 1
        parity = (rows & 1).astype(np.int64)
        dloc = edge_dst[lo:hi].astype(np.int64) - k * NPC
        val_k = edge_val[lo:hi]
        w = dloc // WIN
        streams = []
        for s in range(2):
            m = parity == s
            counts[s, k] = np.bincount(w[m], minlength=nwin)
            streams.append((pair[m], dloc[m], val_k[m], w[m]))
        per_core.append(streams)

    # chunks per (stream, window): max over cores
    cnt = [np.maximum(-(-counts[s].max(axis=0) // 128), 1) for s in range(2)]
    out = {}
    for s in range(2):
        coff = np.zeros(nwin, np.int64)
        coff[1:] = np.cumsum(cnt[s])[:-1]
        nch = int(cnt[s].sum())
        idx = np.zeros((N_CORES, 128, nch), np.int16)
        dcol = np.zeros((N_CORES, 128, nch), np.float32)
        val = np.zeros((N_CORES, 128, nch), np.float32)
        for k in range(N_CORES):
            pairs, dloc, vals, ws = per_core[k][s]
            win_start = np.zeros(nwin, np.int64)
            win_start[1:] = np.cumsum(counts[s, k])[:-1]
            within = np.arange(len(pairs), dtype=np.int64) - win_start[ws]
            slot_c = coff[ws] + within // 128
            slot_p = within % 128
            idx[k, slot_p, slot_c] = pairs
            dcol[k, slot_p, slot_c] = (dloc - ws * WIN).astype(np.float32)
            val[k, slot_p, slot_c] = vals
        out[s] = (idx, dcol, val)
    return (
        out,
        [list(map(int, cnt[0])), list(map(int, cnt[1]))],
        nwin,
        used,
        n_sup_chunks,
    )


def _dma_gather_raw(
    g, out_ap, in_ap, idxs_ap, num_idxs, elem_size, elem_step, queue_num=0
):
    """nc.gpsimd.dma_gather without the elem_size_bytes % 256 restriction.

    That API assert conflates the descriptor *stride* granularity (which IS
    256 bytes: stride_bytes_256 below) with the per-descriptor read length.
    Here elem_size (read length) may be any size while elem_step (stride)
    must still be a multiple of 256 bytes. Mirrors the tail of
    BassGpSimd.dma_gather for the non-transpose HBM-source path.
    """
    import concourse.mybir as mybir
    from concourse._compat import exact_div

    assert idxs_ap.dtype == mybir.dt.int16
    assert in_ap.dtype == out_ap.dtype
    assert in_ap.ap[0][0] == elem_step
    stride_bytes = elem_step * mybir.dt.size(in_ap.dtype)
    stride_bytes_256 = exact_div(stride_bytes, 256)
    assert stride_bytes_256 < 256
    _in_ap = g.lower_ap_dma(in_ap, for_custom_bir_dma=True)
    _idxs_ap = g.lower_ap(idxs_ap)
    _out_ap = g.lower_ap(out_ap)
    return g.add_instruction(
        mybir.InstDMAGatherAnt(
            name=g.bass.get_next_instruction_name(),
            ins=[
                *_in_ap,
                _idxs_ap,
                g.lower_val_access(g.to_reg(num_idxs)),
            ],
            outs=[_out_ap],
            transpose=False,
            num_idxs=num_idxs,
            elem_size=elem_size,
            stride_bytes_256=stride_bytes_256,
            gen_mode=0,
            single_packet=False,
            queue_num=queue_num,
            sbuf_tokens_per_rank=0,
            sbuf_free_dim_per_rank=0,
            sbuf_free_dim_pad_per_rank=0,
            sbuf_byte_offset=0,
        )
    )


def _build_program(cnt, nwin, n_sup_chunks):
    import concourse.bacc as bacc
    import concourse.mybir as mybir
    from concourse import tile

    F32 = mybir.dt.float32
    BF16 = mybir.dt.bfloat16
    I16 = mybir.dt.int16
    nch = [sum(cnt[0]), sum(cnt[1])]
    coff = [np.concatenate([[0], np.cumsum(cnt[s])[:-1]]) for s in range(2)]
    n_sup_rows = n_sup_chunks * 128
    n_pairs = n_sup_rows // 2

    nc = bacc.Bacc(None, target_bir_lowering=False, debug=False)
    xT_d = nc.dram_tensor("xT", [D_IN, n_sup_rows], BF16, kind="ExternalInput")
    wT_d = nc.dram_tensor("wT", [D_IN, D_OUT], BF16, kind="ExternalInput")
    ones_d = nc.dram_tensor("ones", [1, 128], BF16, kind="ExternalInput")
    b3_d = nc.dram_tensor("b3", [1, D_OUT], BF16, kind="ExternalInput")
    iota_d = nc.dram_tensor("iota", [128, WIN], BF16, kind="ExternalInput")
    idx_d = [
        nc.dram_tensor(f"idx{s}", [128, nch[s] * 8], I16, kind="ExternalInput")
        for s in range(2)
    ]
    dcol_d = [
        nc.dram_tensor(f"dcol{s}", [128, nch[s]], F32, kind="ExternalInput")
        for s in range(2)
    ]
    val_d = [
        nc.dram_tensor(f"val{s}", [128, nch[s]], F32, kind="ExternalInput")
        for s in range(2)
    ]
    sup_d = nc.dram_tensor("support", [n_sup_rows, D_OUT], BF16, kind="Internal")
    y_d = nc.dram_tensor("y", [128, nwin, D_OUT], F32, kind="ExternalOutput")

    # pair-strided views of the support table: stream s gathers 64 bf16
    # elements at byte offset 128*s within each 256-byte (row-pair) stride
    sup_pairs = sup_d.rearrange("(q t) f -> q (t f)", t=2)

    # phase-2 chunk consumption order (window-major, stream 0 then 1)
    order = []
    for w in range(nwin):
        for s in range(2):
            for i in range(cnt[s][w]):
                order.append((s, int(coff[s][w]) + i))
    slot_of = {sc: j for j, sc in enumerate(order)}

    with tile.TileContext(nc) as tc:
        with (
            tc.tile_pool(name="const", bufs=1) as cpool,
            tc.tile_pool(name="stage", bufs=1) as stpool,
            tc.tile_pool(name="xt", bufs=XBUFS) as xpool,
            tc.tile_pool(name="gath", bufs=GBUFS) as gpool,
            tc.tile_pool(name="smat", bufs=SBUFS) as spool,
            tc.tile_pool(name="flush", bufs=FBUFS) as fpool,
            tc.tile_pool(name="psum1", bufs=P1BUFS, space="PSUM") as pspool1,
            tc.tile_pool(name="psum2", bufs=P2BUFS, space="PSUM") as pspool2,
        ):
            # constants / metadata
            wt_t = cpool.tile([128, 2, D_OUT], BF16)
            ones_t = cpool.tile([1, 128], BF16)
            b3_t = cpool.tile([1, D_OUT], BF16)
            iota_t = cpool.tile([128, WIN], BF16)
            idx_t = [
                cpool.tile([128, nch[s] * 8], I16, name=f"idx{s}t", tag=f"idx{s}")
                for s in range(2)
            ]
            dcol_t = [
                cpool.tile([128, nch[s]], F32, name=f"dcol{s}t", tag=f"dcol{s}")
                for s in range(2)
            ]
            val_t = [
                cpool.tile([128, nch[s]], F32, name=f"val{s}t", tag=f"val{s}")
                for s in range(2)
            ]
            wT_v = wT_d.rearrange("(kk p) f -> p kk f", p=128)
            nc.sync.dma_start(out=wt_t[:], in_=wT_v)
            nc.sync.dma_start(out=ones_t[:], in_=ones_d[:])
            nc.sync.dma_start(out=b3_t[:], in_=b3_d[:])
            nc.sync.dma_start(out=iota_t[:], in_=iota_d[:])
            for s in range(2):
                nc.sync.dma_start(out=idx_t[s][:], in_=idx_d[s][:])
                nc.sync.dma_start(out=dcol_t[s][:], in_=dcol_d[s][:])
                nc.sync.dma_start(out=val_t[s][:], in_=val_d[s][:])

            # pre-build the LAST NPRE S-matrices into a static arena: they
            # depend only on metadata, so the DVE does this work under the
            # shadow of phase 1's DMA/PE/Act activity. Covering the tail
            # windows lets the post-last-gather drain run at matmul speed
            # instead of S-build -> matmul sem ping-pong.
            npre = min(NPRE, len(order))
            pre_base = len(order) - npre
            s_pre = cpool.tile([128, npre, WIN], BF16)
            for j in range(pre_base, len(order)):
                s, c = order[j]
                nc.vector.tensor_scalar(
                    s_pre[:, j - pre_base, :], iota_t[:],
                    dcol_t[s][:, c : c + 1], val_t[s][:, c : c + 1],
                    op0=mybir.AluOpType.is_equal,
                    op1=mybir.AluOpType.mult,
                )

            # phase 1: support = x @ W.T + b, written to HBM per group (bf16)
            done = 0
            while done < n_sup_chunks:
                g = min(MG_CHUNKS, n_sup_chunks - done)
                xt_t = xpool.tile([128, 2, g * 128], BF16, tag="xt")
                sup_grp = stpool.tile(
                    [128, g, D_OUT], BF16, tag="supgrp", bufs=SGBUFS
                )
                for kk in range(2):
                    # issue xT loads from the Act queue: SP's 565ns/DMA
                    # sequencer config time would serialize phase 1
                    nc.scalar.dma_start(
                        out=xt_t[:, kk, :],
                        in_=xT_d[kk * 128 : (kk + 1) * 128,
                                 done * 128 : (done + g) * 128],
                    )
                for j0 in range(0, g, B1):
                    b1 = min(B1, g - j0)
                    p1 = pspool1.tile([128, B1, D_OUT], F32, name="p1", tag="ps1")
                    for jj in range(b1):
                        j = j0 + jj
                        for kk in range(2):
                            nc.tensor.matmul(
                                p1[:, jj, :],
                                xt_t[:, kk, j * 128 : (j + 1) * 128],
                                wt_t[:, kk, :],
                                start=(kk == 0),
                                stop=False,
                            )
                        # bias via rank-1 ones-row matmul
                        nc.tensor.matmul(
                            p1[:, jj, :], ones_t[:], b3_t[:],
                            start=False, stop=True,
                        )
                    # batched PSUM->SBUF copy on the Activation engine
                    nc.scalar.activation(
                        sup_grp[:, j0 : j0 + b1, :], p1[:, :b1, :],
                        mybir.ActivationFunctionType.Copy,
                    )
                off = done * 128
                gv = sup_d[off : off + g * 128, :].rearrange(
                    "(p ch) f -> p ch f", ch=g
                )
                nc.sync.dma_start(out=gv, in_=sup_grp[:])
                done += g

            # phase 2: parity-stream gathers + S-matmul segment sum
            out_stage = stpool.tile([128, nwin, D_OUT], F32)
            g_tiles = {0: {}, 1: {}}

            def ensure_gather(s, c):
                grp = c // GC
                if grp not in g_tiles[s]:
                    lo = grp * GC
                    hi = min(lo + GC, nch[s])
                    n_idx = (hi - lo) * 128
                    gt = gpool.tile(
                        [128, hi - lo, D_OUT], BF16, name=f"g{s}_{grp}", tag=f"G{s}"
                    )
                    _dma_gather_raw(
                        nc.gpsimd,
                        gt[:, :, :],
                        sup_pairs[:, s * D_OUT : (s + 1) * D_OUT],
                        idx_t[s][:, lo * 8 : hi * 8],
                        n_idx,
                        D_OUT,
                        2 * D_OUT,
                    )
                    g_tiles[s][grp] = gt
                return g_tiles[s][grp], c - grp * GC

            s_state = [None, SGRP]  # rolling S tile, next free slice

            def next_s_slice(s, c):
                j = slot_of[(s, c)]
                if j >= pre_base:
                    return s_pre[:, j - pre_base, :], True
                if s_state[1] == SGRP:
                    s_state[0] = spool.tile(
                        [128, SGRP, WIN], BF16, name="s_grp", tag="S"
                    )
                    s_state[1] = 0
                sl = s_state[0][:, s_state[1], :]
                s_state[1] += 1
                return sl, False

            ydone = 0
            for w in range(nwin):
                p2 = pspool2.tile([128, D_OUT], F32, name="p2", tag="ps2")
                n_acc = cnt[0][w] + cnt[1][w]
                assert n_acc >= 1
                j = 0
                for s in range(2):
                    for i in range(cnt[s][w]):
                        c = int(coff[s][w]) + i
                        gt, off = ensure_gather(s, c)
                        s_t, prebuilt = next_s_slice(s, c)
                        if not prebuilt:
                            nc.vector.tensor_scalar(
                                s_t, iota_t[:],
                                dcol_t[s][:, c : c + 1], val_t[s][:, c : c + 1],
                                op0=mybir.AluOpType.is_equal,
                                op1=mybir.AluOpType.mult,
                            )
                        nc.tensor.matmul(
                            p2[:],
                            s_t,
                            gt[:, off, :],
                            start=(j == 0),
                            stop=(j == n_acc - 1),
                        )
                        j += 1
                tmp = fpool.tile([128, D_OUT], F32, tag="tmp")
                nc.scalar.activation(
                    tmp[:], p2[:], mybir.ActivationFunctionType.Copy,
                    scale=NEG_SLOPE,
                )
                nc.vector.tensor_tensor(
                    out_stage[:, w, :], tmp[:], p2[:], op=mybir.AluOpType.max
                )
                # flush finished windows to HBM in YSPLIT pieces so the
                # final DMA only waits on the tail windows
                if w + 1 == (ydone + 1) * nwin // YSPLIT:
                    lo = ydone * nwin // YSPLIT
                    nc.sync.dma_start(
                        out=y_d[:, lo : w + 1, :],
                        in_=out_stage[:, lo : w + 1, :],
                    )
                    ydone += 1
    nc.compile()
    return nc


LAST_RESULTS = None  # BassKernelResults of the most recent run (for profiling)
LAST_NC = None  # compiled Bass module of the most recent run


def kernel(x, W, b, edge_src, edge_dst, edge_val):
    global LAST_RESULTS, LAST_NC
    from concourse import bass_utils

    x = np.asarray(x)
    W = np.asarray(W)
    b = np.asarray(b)
    edge_src = np.asarray(edge_src)
    edge_dst = np.asarray(edge_dst)
    edge_val = np.asarray(edge_val)

    streams, cnt, nwin, used, n_sup_chunks = _build_schedule(
        edge_src, edge_dst, edge_val
    )
    n_sup_rows = n_sup_chunks * 128

    xTb = np.ascontiguousarray(x.T).astype(ml_dtypes.bfloat16)
    wT = np.ascontiguousarray(W.T).astype(ml_dtypes.bfloat16)
    ones = np.ones((1, 128), ml_dtypes.bfloat16)
    b3 = b.astype(ml_dtypes.bfloat16).reshape(1, D_OUT)
    iota = np.tile(
        np.arange(WIN, dtype=np.float32), (128, 1)
    ).astype(ml_dtypes.bfloat16)

    nc = _build_program(cnt, nwin, n_sup_chunks)
    LAST_NC = nc

    shared = {"wT": wT, "ones": ones, "b3": b3, "iota": iota}
    in_maps = []
    for k in range(N_CORES):
        m = dict(shared)
        xT = np.zeros((D_IN, n_sup_rows), ml_dtypes.bfloat16)
        xT[:, : len(used[k])] = xTb[:, used[k]]
        m["xT"] = xT
        for s in range(2):
            idx, dcol, val = streams[s]
            nch_s = idx.shape[-1]
            # wrap each chunk's 128 indices: j -> [j % 16, j // 16]
            iw = _wrap16(idx[k].T.reshape(nch_s * 128))  # [16, nch*8]
            m[f"idx{s}"] = np.ascontiguousarray(np.tile(iw, (8, 1)))
            m[f"dcol{s}"] = dcol[k]
            m[f"val{s}"] = val[k]
        in_maps.append(m)

    res = None
    for attempt in range(3):
        try:
            res = bass_utils.run_bass_kernel_spmd(
                nc, in_maps, core_ids=list(range(N_CORES))
            )
            break
        except Exception:
            # Transient NRT/axon execution failures have been observed; the
            # device recovers on a fresh dispatch. Re-raise on the last try.
            if attempt == 2:
                raise
    LAST_RESULTS = res
    out = np.concatenate(
        [
            res.results[k]["y"].transpose(1, 0, 2).reshape(-1, D_OUT)[:NPC]
            for k in range(N_CORES)
        ],
        axis=0,
    )
    return out.astype(np.float32)


if __name__ == "__main__":
    pass


# revision 40
# speedup vs baseline: 1.6058x; 1.0333x over previous
"""GCN layer kernel for Trainium2 (8 NeuronCores, SPMD).

Computes: support = x @ W.T + b; agg[d] = sum_{e: dst[e]=d} val[e] * support[src[e]];
out = leaky_relu(agg, 0.2).

Strategy (dst-sharded, 6250 dst nodes / ~100K edges per core):
  Phase 1: every core computes the full support [N, 64] in BF16 (x fed as
           pre-transposed bf16) and writes it to HBM in per-group DMAs.
           The HBM layout is a group-aligned partition-major permutation
           (see _row_of_node) so each write is contiguous per partition.
  Phase 2: edges are packed on the host into 128-edge chunks grouped under
           128-dst windows. Chunk counts per (window, parity-stream) are the
           max over cores (the program is shared SPMD). The support table is
           viewed as [25024 pairs x 256B]; a raw InstDMAGatherAnt (emitted
           directly to allow elem_size_bytes=128 < the API's 256B minimum;
           the 256B rule is the *stride* granularity, which we satisfy with
           elem_step=128 bf16) gathers the 64-feature bf16 row of each edge:
           stream 0 reads pair offset +0 (even rows), stream 1 offset +64
           elements (odd rows), so int16 pair indices stay < 32768. Per
           chunk: a one-hot "S" matrix ([128 edges x 128 dsts] bf16,
           S[p, dcol[p]] = val[p]) built on the vector engine with one fused
           is_equal*val tensor_scalar op, then a PE matmul S.T @ G
           accumulated in PSUM over the window. Leaky relu is applied at
           PSUM flush; output is staged in SBUF and written with one DMA,
           de-permuted on the host.
"""

import math

import numpy as np
import ml_dtypes

N_NODES = 50000
N_EDGES = 800000
D_IN = 256
D_OUT = 64
NEG_SLOPE = 0.2
N_CORES = 8
NPC = N_NODES // N_CORES  # dst nodes per core
WIN = 128  # dst window width (S matrix columns / psum partitions)
MG_CHUNKS = 16  # phase-1 node chunks per xT load group
GC = 48  # chunks per dma_gather call
FG = 16  # chunks in the first gather call per stream
GBUFS = 3  # gather tile buffer depth (per stream)
SGRP = 8  # S-matrix chunks per tile (amortizes tile-pool sync)
SBUFS = 6  # S-matrix buffer depth
NPRE = 408  # S-matrix chunks pre-built into a static arena during phase 1
B1 = 16  # phase-1 node chunks per PSUM tile / Act copy
XBUFS = 3  # xT tile buffer depth
P1BUFS = 2  # phase-1 PSUM pool depth ([128, B1*64] f32 tiles)
P2BUFS = 4  # phase-2 PSUM pool depth ([128, 64] f32 tiles)
FBUFS = 4  # flush tmp buffer depth
SGBUFS = 2  # phase-1 support group buffer depth
YSPLIT = 12  # output DMA pieces


def _wrap16(a):
    """[?, n] -> int16 indices wrapped: position j -> [j % 16, j // 16]."""
    n = a.shape[-1]
    assert n % 16 == 0
    return np.ascontiguousarray(
        a.reshape(*a.shape[:-1], n // 16, 16).swapaxes(-1, -2)
    )


def _row_of_node(n_sup_chunks):
    """Group-aligned partition-major support row permutation.

    Within each MG_CHUNKS-chunk phase-1 group, node n maps to row
    base + (n % 128) * gsz + (n // 128 - first_chunk), so every group's
    write is contiguous per partition.
    """
    row = np.empty(n_sup_chunks * 128, np.int64)
    done = 0
    while done < n_sup_chunks:
        g = min(MG_CHUNKS, n_sup_chunks - done)
        nodes = np.arange(done * 128, (done + g) * 128)
        row[nodes] = done * 128 + (nodes % 128) * g + (nodes // 128 - done)
        done += g
    return row


def _build_schedule(edge_src, edge_dst, edge_val):
    """Pack edges into per-core per-parity-stream slot arrays.

    Returns (streams, cnt) where streams[s] = (idx, dcol, val) arrays of
    shape [N_CORES, 128, nch_s] (idx = int16 pair index) and cnt[s] is the
    per-window chunk count list (shared across cores: max over cores).
    """
    nwin = -(-NPC // WIN)  # windows per core
    core_bounds = np.searchsorted(edge_dst, np.arange(0, N_NODES + 1, NPC))

    # per-core used source nodes: each core's support table only holds the
    # ~43K nodes its edges reference (compacted, then group-permuted)
    used = [
        np.unique(edge_src[int(core_bounds[k]) : int(core_bounds[k + 1])])
        for k in range(N_CORES)
    ]
    n_sup_chunks = -(-max(len(u) for u in used) // 128)
    row_perm = _row_of_node(n_sup_chunks)

    per_core = []
    counts = np.zeros((2, N_CORES, nwin), np.int64)
    for k in range(N_CORES):
        lo, hi = int(core_bounds[k]), int(core_bounds[k + 1])
        rows = row_perm[np.searchsorted(used[k], edge_src[lo:hi])]
        pair = rows >> 1
        parity = (rows & 1).astype(np.int64)
        dloc = edge_dst[lo:hi].astype(np.int64) - k * NPC
        val_k = edge_val[lo:hi]
        w = dloc // WIN
        streams = []
        for s in range(2):
            m = parity == s
            counts[s, k] = np.bincount(w[m], minlength=nwin)
            streams.append((pair[m], dloc[m], val_k[m], w[m]))
        per_core.append(streams)

    # chunks per (stream, window): max over cores
    cnt = [np.maximum(-(-counts[s].max(axis=0) // 128), 1) for s in range(2)]
    out = {}
    for s in range(2):
        coff = np.zeros(nwin, np.int64)
        coff[1:] = np.cumsum(cnt[s])[:-1]
        nch = int(cnt[s].sum())
        idx = np.zeros((N_CORES, 128, nch), np.int16)
        dcol = np.zeros((N_CORES, 128, nch), np.float32)
        val = np.zeros((N_CORES, 128, nch), np.float32)
        for k in range(N_CORES):
            pairs, dloc, vals, ws = per_core[k][s]
            win_start = np.zeros(nwin, np.int64)
            win_start[1:] = np.cumsum(counts[s, k])[:-1]
            within = np.arange(len(pairs), dtype=np.int64) - win_start[ws]
            slot_c = coff[ws] + within // 128
            slot_p = within % 128
            idx[k, slot_p, slot_c] = pairs
            dcol[k, slot_p, slot_c] = (dloc - ws * WIN).astype(np.float32)
            val[k, slot_p, slot_c] = vals
        out[s] = (idx, dcol, val)
    return (
        out,
        [list(map(int, cnt[0])), list(map(int, cnt[1]))],
        nwin,
        used,
        n_sup_chunks,
    )


def _dma_gather_raw(
    g, out_ap, in_ap, idxs_ap, num_idxs, elem_size, elem_step, queue_num=0
):
    """nc.gpsimd.dma_gather without the elem_size_bytes % 256 restriction.

    That API assert conflates the descriptor *stride* granularity (which IS
    256 bytes: stride_bytes_256 below) with the per-descriptor read length.
    Here elem_size (read length) may be any size while elem_step (stride)
    must still be a multiple of 256 bytes. Mirrors the tail of
    BassGpSimd.dma_gather for the non-transpose HBM-source path.
    """
    import concourse.mybir as mybir
    from concourse._compat import exact_div

    assert idxs_ap.dtype == mybir.dt.int16
    assert in_ap.dtype == out_ap.dtype
    assert in_ap.ap[0][0] == elem_step
    stride_bytes = elem_step * mybir.dt.size(in_ap.dtype)
    stride_bytes_256 = exact_div(stride_bytes, 256)
    assert stride_bytes_256 < 256
    _in_ap = g.lower_ap_dma(in_ap, for_custom_bir_dma=True)
    _idxs_ap = g.lower_ap(idxs_ap)
    _out_ap = g.lower_ap(out_ap)
    return g.add_instruction(
        mybir.InstDMAGatherAnt(
            name=g.bass.get_next_instruction_name(),
            ins=[
                *_in_ap,
                _idxs_ap,
                g.lower_val_access(g.to_reg(num_idxs)),
            ],
            outs=[_out_ap],
            transpose=False,
            num_idxs=num_idxs,
            elem_size=elem_size,
            stride_bytes_256=stride_bytes_256,
            gen_mode=0,
            single_packet=False,
            queue_num=queue_num,
            sbuf_tokens_per_rank=0,
            sbuf_free_dim_per_rank=0,
            sbuf_free_dim_pad_per_rank=0,
            sbuf_byte_offset=0,
        )
    )


def _build_program(cnt, nwin, n_sup_chunks):
    import concourse.bacc as bacc
    import concourse.mybir as mybir
    from concourse import tile

    F32 = mybir.dt.float32
    BF16 = mybir.dt.bfloat16
    I16 = mybir.dt.int16
    nch = [sum(cnt[0]), sum(cnt[1])]
    coff = [np.concatenate([[0], np.cumsum(cnt[s])[:-1]]) for s in range(2)]
    n_sup_rows = n_sup_chunks * 128
    n_pairs = n_sup_rows // 2

    nc = bacc.Bacc(None, target_bir_lowering=False, debug=False)
    xT_d = nc.dram_tensor("xT", [D_IN, n_sup_rows], BF16, kind="ExternalInput")
    wT_d = nc.dram_tensor("wT", [D_IN, D_OUT], BF16, kind="ExternalInput")
    ones_d = nc.dram_tensor("ones", [1, 128], BF16, kind="ExternalInput")
    b3_d = nc.dram_tensor("b3", [1, D_OUT], BF16, kind="ExternalInput")
    iota_d = nc.dram_tensor("iota", [128, WIN], BF16, kind="ExternalInput")
    idx_d = [
        nc.dram_tensor(f"idx{s}", [128, nch[s] * 8], I16, kind="ExternalInput")
        for s in range(2)
    ]
    dcol_d = [
        nc.dram_tensor(f"dcol{s}", [128, nch[s]], F32, kind="ExternalInput")
        for s in range(2)
    ]
    val_d = [
        nc.dram_tensor(f"val{s}", [128, nch[s]], F32, kind="ExternalInput")
        for s in range(2)
    ]
    sup_d = nc.dram_tensor("support", [n_sup_rows, D_OUT], BF16, kind="Internal")
    y_d = nc.dram_tensor("y", [128, nwin, D_OUT], BF16, kind="ExternalOutput")

    # pair-strided views of the support table: stream s gathers 64 bf16
    # elements at byte offset 128*s within each 256-byte (row-pair) stride
    sup_pairs = sup_d.rearrange("(q t) f -> q (t f)", t=2)

    # phase-2 chunk consumption order (window-major, stream 0 then 1)
    order = []
    for w in range(nwin):
        for s in range(2):
            for i in range(cnt[s][w]):
                order.append((s, int(coff[s][w]) + i))
    slot_of = {sc: j for j, sc in enumerate(order)}

    with tile.TileContext(nc) as tc:
        with (
            tc.tile_pool(name="const", bufs=1) as cpool,
            tc.tile_pool(name="stage", bufs=1) as stpool,
            tc.tile_pool(name="xt", bufs=XBUFS) as xpool,
            tc.tile_pool(name="gath", bufs=GBUFS) as gpool,
            tc.tile_pool(name="smat", bufs=SBUFS) as spool,
            tc.tile_pool(name="flush", bufs=FBUFS) as fpool,
            tc.tile_pool(name="psum1", bufs=P1BUFS, space="PSUM") as pspool1,
            tc.tile_pool(name="psum2", bufs=P2BUFS, space="PSUM") as pspool2,
        ):
            # constants / metadata
            wt_t = cpool.tile([128, 2, D_OUT], BF16)
            ones_t = cpool.tile([1, 128], BF16)
            b3_t = cpool.tile([1, D_OUT], BF16)
            iota_t = cpool.tile([128, WIN], BF16)
            idx_t = [
                cpool.tile([128, nch[s] * 8], I16, name=f"idx{s}t", tag=f"idx{s}")
                for s in range(2)
            ]
            dcol_t = [
                cpool.tile([128, nch[s]], F32, name=f"dcol{s}t", tag=f"dcol{s}")
                for s in range(2)
            ]
            val_t = [
                cpool.tile([128, nch[s]], F32, name=f"val{s}t", tag=f"val{s}")
                for s in range(2)
            ]
            wT_v = wT_d.rearrange("(kk p) f -> p kk f", p=128)
            nc.sync.dma_start(out=wt_t[:], in_=wT_v)
            nc.sync.dma_start(out=ones_t[:], in_=ones_d[:])
            nc.sync.dma_start(out=b3_t[:], in_=b3_d[:])
            nc.sync.dma_start(out=iota_t[:], in_=iota_d[:])
            for s in range(2):
                nc.sync.dma_start(out=idx_t[s][:], in_=idx_d[s][:])
                nc.sync.dma_start(out=dcol_t[s][:], in_=dcol_d[s][:])
                nc.sync.dma_start(out=val_t[s][:], in_=val_d[s][:])

            # pre-build the LAST NPRE S-matrices into a static arena: they
            # depend only on metadata, so the DVE does this work under the
            # shadow of phase 1's DMA/PE/Act activity. Covering the tail
            # windows lets the post-last-gather drain run at matmul speed
            # instead of S-build -> matmul sem ping-pong.
            npre = min(NPRE, len(order))
            pre_base = len(order) - npre
            s_pre = cpool.tile([128, npre, WIN], BF16)
            for j in range(pre_base, len(order)):
                s, c = order[j]
                nc.vector.tensor_scalar(
                    s_pre[:, j - pre_base, :], iota_t[:],
                    dcol_t[s][:, c : c + 1], val_t[s][:, c : c + 1],
                    op0=mybir.AluOpType.is_equal,
                    op1=mybir.AluOpType.mult,
                )

            # phase 1: support = x @ W.T + b, written to HBM per group (bf16)
            done = 0
            while done < n_sup_chunks:
                g = min(MG_CHUNKS, n_sup_chunks - done)
                xt_t = xpool.tile([128, 2, g * 128], BF16, tag="xt")
                sup_grp = stpool.tile(
                    [128, g, D_OUT], BF16, tag="supgrp", bufs=SGBUFS
                )
                for kk in range(2):
                    # issue xT loads from the Act queue: SP's 565ns/DMA
                    # sequencer config time would serialize phase 1
                    nc.scalar.dma_start(
                        out=xt_t[:, kk, :],
                        in_=xT_d[kk * 128 : (kk + 1) * 128,
                                 done * 128 : (done + g) * 128],
                    )
                for j0 in range(0, g, B1):
                    b1 = min(B1, g - j0)
                    p1 = pspool1.tile([128, B1, D_OUT], F32, name="p1", tag="ps1")
                    for jj in range(b1):
                        j = j0 + jj
                        for kk in range(2):
                            nc.tensor.matmul(
                                p1[:, jj, :],
                                xt_t[:, kk, j * 128 : (j + 1) * 128],
                                wt_t[:, kk, :],
                                start=(kk == 0),
                                stop=False,
                            )
                        # bias via rank-1 ones-row matmul
                        nc.tensor.matmul(
                            p1[:, jj, :], ones_t[:], b3_t[:],
                            start=False, stop=True,
                        )
                    # batched PSUM->SBUF copy on the Activation engine
                    nc.scalar.activation(
                        sup_grp[:, j0 : j0 + b1, :], p1[:, :b1, :],
                        mybir.ActivationFunctionType.Copy,
                    )
                off = done * 128
                gv = sup_d[off : off + g * 128, :].rearrange(
                    "(p ch) f -> p ch f", ch=g
                )
                nc.sync.dma_start(out=gv, in_=sup_grp[:])
                done += g

            # phase 2: parity-stream gathers + S-matmul segment sum
            out_stage = stpool.tile([128, nwin, D_OUT], BF16)
            g_tiles = {0: {}, 1: {}}
            # small first group: its descriptor generation sits on the
            # phase-1 -> phase-2 critical path (it must wait for the last
            # support write), so keep it short
            gbounds = {}
            for s in range(2):
                b = [0, FG]
                while b[-1] < nch[s]:
                    b.append(min(b[-1] + GC, nch[s]))
                gbounds[s] = b

            def grp_of(s, c):
                import bisect

                return bisect.bisect_right(gbounds[s], c) - 1

            def ensure_gather(s, c):
                grp = grp_of(s, c)
                if grp not in g_tiles[s]:
                    lo = gbounds[s][grp]
                    hi = min(gbounds[s][grp + 1], nch[s])
                    n_idx = (hi - lo) * 128
                    gt = gpool.tile(
                        [128, hi - lo, D_OUT], BF16, name=f"g{s}_{grp}", tag=f"G{s}"
                    )
                    _dma_gather_raw(
                        nc.gpsimd,
                        gt[:, :, :],
                        sup_pairs[:, s * D_OUT : (s + 1) * D_OUT],
                        idx_t[s][:, lo * 8 : hi * 8],
                        n_idx,
                        D_OUT,
                        2 * D_OUT,
                    )
                    g_tiles[s][grp] = gt
                return g_tiles[s][grp], c - gbounds[s][grp]

            s_state = [None, SGRP]  # rolling S tile, next free slice

            def next_s_slice(s, c):
                j = slot_of[(s, c)]
                if j >= pre_base:
                    return s_pre[:, j - pre_base, :], True
                if s_state[1] == SGRP:
                    s_state[0] = spool.tile(
                        [128, SGRP, WIN], BF16, name="s_grp", tag="S"
                    )
                    s_state[1] = 0
                sl = s_state[0][:, s_state[1], :]
                s_state[1] += 1
                return sl, False

            ydone = 0
            for w in range(nwin):
                p2 = pspool2.tile([128, D_OUT], F32, name="p2", tag="ps2")
                n_acc = cnt[0][w] + cnt[1][w]
                assert n_acc >= 1
                j = 0
                for s in range(2):
                    for i in range(cnt[s][w]):
                        c = int(coff[s][w]) + i
                        gt, off = ensure_gather(s, c)
                        s_t, prebuilt = next_s_slice(s, c)
                        if not prebuilt:
                            nc.vector.tensor_scalar(
                                s_t, iota_t[:],
                                dcol_t[s][:, c : c + 1], val_t[s][:, c : c + 1],
                                op0=mybir.AluOpType.is_equal,
                                op1=mybir.AluOpType.mult,
                            )
                        nc.tensor.matmul(
                            p2[:],
                            s_t,
                            gt[:, off, :],
                            start=(j == 0),
                            stop=(j == n_acc - 1),
                        )
                        j += 1
                nc.scalar.activation(
                    out_stage[:, w, :], p2[:],
                    mybir.ActivationFunctionType.Prelu, alpha=NEG_SLOPE,
                )
                # flush finished windows to HBM in YSPLIT pieces so the
                # final DMA only waits on the tail windows
                if w + 1 == (ydone + 1) * nwin // YSPLIT:
                    lo = ydone * nwin // YSPLIT
                    nc.sync.dma_start(
                        out=y_d[:, lo : w + 1, :],
                        in_=out_stage[:, lo : w + 1, :],
                    )
                    ydone += 1
    nc.compile()
    return nc


LAST_RESULTS = None  # BassKernelResults of the most recent run (for profiling)
LAST_NC = None  # compiled Bass module of the most recent run


def kernel(x, W, b, edge_src, edge_dst, edge_val):
    global LAST_RESULTS, LAST_NC
    from concourse import bass_utils

    x = np.asarray(x)
    W = np.asarray(W)
    b = np.asarray(b)
    edge_src = np.asarray(edge_src)
    edge_dst = np.asarray(edge_dst)
    edge_val = np.asarray(edge_val)

    streams, cnt, nwin, used, n_sup_chunks = _build_schedule(
        edge_src, edge_dst, edge_val
    )
    n_sup_rows = n_sup_chunks * 128

    xTb = np.ascontiguousarray(x.T).astype(ml_dtypes.bfloat16)
    wT = np.ascontiguousarray(W.T).astype(ml_dtypes.bfloat16)
    ones = np.ones((1, 128), ml_dtypes.bfloat16)
    b3 = b.astype(ml_dtypes.bfloat16).reshape(1, D_OUT)
    iota = np.tile(
        np.arange(WIN, dtype=np.float32), (128, 1)
    ).astype(ml_dtypes.bfloat16)

    nc = _build_program(cnt, nwin, n_sup_chunks)
    LAST_NC = nc

    shared = {"wT": wT, "ones": ones, "b3": b3, "iota": iota}
    in_maps = []
    for k in range(N_CORES):
        m = dict(shared)
        xT = np.zeros((D_IN, n_sup_rows), ml_dtypes.bfloat16)
        xT[:, : len(used[k])] = xTb[:, used[k]]
        m["xT"] = xT
        for s in range(2):
            idx, dcol, val = streams[s]
            nch_s = idx.shape[-1]
            # wrap each chunk's 128 indices: j -> [j % 16, j // 16]
            iw = _wrap16(idx[k].T.reshape(nch_s * 128))  # [16, nch*8]
            m[f"idx{s}"] = np.ascontiguousarray(np.tile(iw, (8, 1)))
            m[f"dcol{s}"] = dcol[k]
            m[f"val{s}"] = val[k]
        in_maps.append(m)

    res = None
    for attempt in range(3):
        try:
            res = bass_utils.run_bass_kernel_spmd(
                nc, in_maps, core_ids=list(range(N_CORES))
            )
            break
        except Exception:
            # Transient NRT/axon execution failures have been observed; the
            # device recovers on a fresh dispatch. Re-raise on the last try.
            if attempt == 2:
                raise
    LAST_RESULTS = res
    out = np.concatenate(
        [
            np.asarray(res.results[k]["y"], dtype=np.float32)
            .transpose(1, 0, 2).reshape(-1, D_OUT)[:NPC]
            for k in range(N_CORES)
        ],
        axis=0,
    )
    return out.astype(np.float32)


if __name__ == "__main__":
    pass
